# revision 13
# baseline (speedup 1.0000x reference)
"""Trainium2 Bass kernel for pairwise MI discriminator (gnn_message_passing).

Math (see reference):
  ls   = gather(locals_, idx_t)                       [B, MT*LD]
  globals_mi[i,j]   = MLPg(concat(ls[i], globals_[j]))          [B,B]
  locals_mi[t,i,j]  = MLPl(concat(locals_[i,t], globals_[j]))   [T,B,B]
where each MLP is Linear+ReLU, Linear+ReLU, Linear(->1).

Key factorization: layer 1 of each MLP splits over the concat:
  h1[i,j] = relu(A[i] + Bg[j] + b1),  A = ls @ W1[:640], Bg = globals_ @ W1[640:]
so the O(B^2) first-layer matmul collapses to O(B).

Sharding: rows i are split across 8 cores (12 rows each); globals_ and
weights replicated. Each core computes its [12,96] slice of globals_mi and
[96,12,48] slice of locals_mi ((j,i,t) order), host reassembles.

On-chip layout ("features on partitions"): activations are kept transposed
[128=hidden, cols] so layers 2/3 are natural matmuls with stationary weights.
Layer 3 (128->1) packs its per-column scalar outputs into PSUM partition rows
via a diagonal-w3 stationary operand + PSUM accumulation + 4-way column-group
tile_position concurrency, so one PSUM bank collects 64 j-columns of output
and is copied out in a single wide op.
"""

import sys

if "/opt/trn_rl_repo" not in sys.path:
    sys.path.insert(0, "/opt/trn_rl_repo")

import numpy as np

B, T, LD, GD, MT, H = 96, 48, 64, 64, 10, 128
NCORES = 8
BI = B // NCORES          # 12 rows of i per core
NC_COLS = BI * T          # 576 columns (i,t) per j
NPAIR = B // 2            # 48 j-pairs
Q = NC_COLS // 2          # 288 = quarter of a j-pair's 1152 cols

# engine split tables: combine per j (V=vector, G=gpsimd, A=scalar/ACT),
# psum-out per pair (A=scalar/ACT, V=vector)
CB_SPLIT = "V" * 48
PO_SPLIT = "A"

_CACHE = {}


def _build_bass():
    import concourse.mybir as mybir
    import concourse.tile as tile
    from concourse import bacc

    f32 = mybir.dt.float32
    f32r = mybir.dt.float32r
    bf16 = mybir.dt.bfloat16
    fp16 = mybir.dt.float16
    Alu = mybir.AluOpType
    Act = mybir.ActivationFunctionType

    nc = bacc.Bacc("TRN2", target_bir_lowering=False, debug=False)

    def din(name, shape):
        return nc.dram_tensor(name, shape, f32, kind="ExternalInput").ap()

    lsT = din("lsT", [MT * LD, BI])        # sampled locals, transposed
    locT = din("locT", [128, NC_COLS])     # locals rows this core, T-major, rows 64+ zero
    gloTa = din("gloTa", [128, B])         # globals_.T in rows 0..64, rest zero
    gloTb = din("gloTb", [128, B])         # globals_.T in rows 64..128, rest zero
    gw1a = din("gw1a", [MT * LD, H])
    gw1bp = din("gw1bp", [128, H])         # gw1[640:] zero-padded to 128 rows
    gw2d = din("gw2", [H, H])
    lw1d = din("lw1", [H, H])
    lw2d = din("lw2", [H, H])
    w3dg = din("w3dg", [H, 32 * 32])       # [h, r*32+m]: lw3[h] iff m==r
    gw3d = din("gw3d", [H, 32])            # [h, m]: gw3[h] iff m==0
    gb1 = din("gb1", [H, 1])
    gb2 = din("gb2", [H, 1])
    lb1 = din("lb1", [H, 1])
    lb2 = din("lb2", [H, 1])

    gmi = nc.dram_tensor("gmi", [BI * B], f32, kind="ExternalOutput").ap()
    lmi = nc.dram_tensor("lmi", [B, NC_COLS], f32, kind="ExternalOutput").ap()

    with tile.TileContext(nc) as tc:
        with (
            tc.tile_pool(name="const", bufs=1) as const,
            tc.tile_pool(name="h1p", bufs=4) as h1p,
            tc.tile_pool(name="h2p", bufs=4) as h2p,
            tc.tile_pool(name="l3p", bufs=2) as l3p,
            tc.tile_pool(name="ps2", bufs=2, space="PSUM") as ps2,
            tc.tile_pool(name="ps3", bufs=2, space="PSUM") as ps3,
        ):
            # ---- stage 0: load constants -------------------------------
            # spread loads over the per-engine DMA queues so they don't
            # serialize on one HWDGE ring
            _qs = [nc.sync, nc.scalar, nc.gpsimd]
            _qi = [0]

            def load(ap_, shape, rearr=None, **kw):
                t = const.tile(shape, f32, tag=f"c_{ap_.name}")
                eng = _qs[_qi[0] % len(_qs)]
                _qi[0] += 1
                eng.dma_start(out=t[:], in_=ap_ if rearr is None else ap_.rearrange(rearr, **kw))
                return t

            lw1_sb = load(lw1d, [H, H])
            locT_sb = load(locT, [128, NC_COLS])
            gloTb_sb = load(gloTb, [128, B])
            lb1_sb = load(lb1, [H, 1])
            lw2_sb = load(lw2d, [H, H])
            lb2_sb = load(lb2, [H, 1])
            lsT_sb = load(lsT, [128, 5, BI], "(o p) m -> p o m", p=128)
            gw1a_sb = load(gw1a, [128, 5, H], "(o p) m -> p o m", p=128)
            gloTa_sb = load(gloTa, [128, B])
            gw1bp_sb = load(gw1bp, [128, H])
            gw2_sb = load(gw2d, [H, H])
            gb1_sb = load(gb1, [H, 1])
            gb2_sb = load(gb2, [H, 1])
            gw3d_sb = load(gw3d, [H, 32])
            w3dg_sb = load(w3dg, [H, 32, 32], "h (r m) -> h r m", m=32)

            # ---- stage 0: first-layer factor matmuls -------------------
            # AlT [128,576] = lw1[:64].T @ locT  (+lb1)
            ps_al = ps2.tile([128, NC_COLS], f32, tag="ps2")
            nc.tensor.matmul(ps_al[:, 0:512], lw1_sb[:], locT_sb[:, 0:512], start=True, stop=True)
            nc.tensor.matmul(ps_al[:, 512:NC_COLS], lw1_sb[:], locT_sb[:, 512:NC_COLS],
                             start=True, stop=True)
            alT = const.tile([128, NC_COLS], f32)
            nc.vector.tensor_scalar_add(alT[:], ps_al[:], lb1_sb[:, 0:1])

            # BlT [128,96] = lw1[64:].T @ globals_.T
            ps_bl = ps2.tile([128, B], f32, tag="ps2")
            nc.tensor.matmul(ps_bl[:], lw1_sb[:], gloTb_sb[:], start=True, stop=True)
            blT = const.tile([128, B], f32)
            nc.vector.tensor_copy(blT[:], ps_bl[:])

            # AgT [128,12] = gw1[:640].T @ ls.T   (+gb1)
            ps_ag = ps2.tile([128, BI], f32, tag="ps2")
            for k in range(5):
                nc.tensor.matmul(ps_ag[:], gw1a_sb[:, k, :], lsT_sb[:, k, :],
                                 start=(k == 0), stop=(k == 4))
            agT = const.tile([128, BI], f32)
            nc.vector.tensor_scalar_add(agT[:], ps_ag[:], gb1_sb[:, 0:1])

            # BgT [128,96] = gw1[640:].T @ globals_.T
            ps_bg = ps2.tile([128, B], f32, tag="ps2")
            nc.tensor.matmul(ps_bg[:], gw1bp_sb[:], gloTa_sb[:], start=True, stop=True)
            bgT = const.tile([128, B], f32)
            nc.vector.tensor_copy(bgT[:], ps_bg[:])

            def mm_chunks(psum, lhsT, rhs, n):
                # split free dim into <=512 chunks aligned to PSUM banks
                o = 0
                while o < n:
                    w = min(512, n - o)
                    nc.tensor.matmul(psum[:, o:o + w], lhsT, rhs[:, o:o + w],
                                     start=True, stop=True)
                    o += w

            # ---- globals branch ---------------------------------------
            h1g = h1p.tile([128, BI * B], f32, tag="h1")
            for i in range(BI):
                nc.vector.tensor_scalar(h1g[:, i * B:(i + 1) * B], bgT[:],
                                        agT[:, i:i + 1], 0.0, op0=Alu.add, op1=Alu.max)
            ps_g2 = ps2.tile([128, BI * B], f32, tag="ps2")
            mm_chunks(ps_g2, gw2_sb[:], h1g, BI * B)
            h2g = h2p.tile([128, BI * B], f32, tag="h2")
            nc.scalar.activation(h2g[:], ps_g2[:], Act.Relu, bias=gb2_sb[:, 0:1])
            ps_g3 = ps3.tile([128, Q], f32, tag="ps3")
            for g in range(4):
                nc.tensor.matmul(ps_g3[32 * g:32 * (g + 1), :], gw3d_sb[:],
                                 h2g[:, Q * g:Q * (g + 1)], start=True, stop=True,
                                 tile_position=(0, 32 * g))
            g3sb = l3p.tile([128, Q], f32, tag="l3")
            nc.scalar.activation(g3sb[:], ps_g3[:], Act.Copy)
            nc.sync.dma_start(out=gmi.rearrange("(g c) -> g c", g=4),
                              in_=g3sb[0:128:32, :])

            # ---- locals branch: groups of 8 j (4608 cols = 9x512) ------
            # layer-2 psum in [128,1536] chunk tiles (3 banks, 3 MMs of 512);
            # layer-3 outputs all accumulate into ONE psum bank ps_big
            # [128,512]: chunk c (cols [512c,512c+512) of the 55296-col
            # stream) -> row 32*(c%4) + c//4, via diag-w3 lhsT col r=c//4 and
            # col-group tile_position (0, 32*(c%4)).
            NG = B // 8                       # 12 groups
            NCHUNK = 9 * NG                   # 108 global 512-chunks
            ps_big = ps3.tile([128, 512], f32, tag="ps3")
            po_idx = 0
            for G in range(NG):
                h1l = h1p.tile([128, 8 * NC_COLS], f32, tag="h1")
                for jj in range(8):
                    j = 8 * G + jj
                    dst = h1l[:, jj * NC_COLS:(jj + 1) * NC_COLS]
                    if CB_SPLIT[j % len(CB_SPLIT)] == "A":
                        nc.scalar.activation(dst, alT[:], Act.Relu,
                                             bias=blT[:, j:j + 1])
                    else:
                        nc.vector.tensor_scalar(dst, alT[:], blT[:, j:j + 1], 0.0,
                                                op0=Alu.add, op1=Alu.max)
                for ti in range(3):
                    ps_l2 = ps2.tile([128, 1536], f32, tag="ps2")
                    for k in range(3):
                        nc.tensor.matmul(
                            ps_l2[:, 512 * k:512 * (k + 1)], lw2_sb[:],
                            h1l[:, 1536 * ti + 512 * k:1536 * ti + 512 * (k + 1)],
                            start=True, stop=True)
                    h2l = h2p.tile([128, 1536], f32, tag="h2")
                    if PO_SPLIT[po_idx % len(PO_SPLIT)] == "A":
                        nc.scalar.activation(h2l[:], ps_l2[:], Act.Relu,
                                             bias=lb2_sb[:, 0:1])
                    else:
                        nc.vector.tensor_scalar(h2l[:], ps_l2[:], lb2_sb[:, 0:1],
                                                0.0, op0=Alu.add, op1=Alu.max)
                    po_idx += 1
                    for k in range(3):
                        c = 9 * G + 3 * ti + k
                        g, r = c % 4, c // 4
                        nc.tensor.matmul(ps_big[32 * g:32 * (g + 1), :],
                                         w3dg_sb[:, r, :],
                                         h2l[:, 512 * k:512 * (k + 1)],
                                         start=(c < 4), stop=(c >= NCHUNK - 4),
                                         tile_position=(0, 32 * g))
            l3sb = l3p.tile([128, 512], f32, tag="l3")
            nc.scalar.activation(l3sb[:], ps_big[:], Act.Copy)
            # row 32*g+r holds cols [512*(4r+g), +512) of the flat (j,i,t) out
            lmi_rx = lmi.rearrange("a b -> (a b)").rearrange("(r c) -> r c", c=2048)
            for g in range(4):
                nc.sync.dma_start(out=lmi_rx[:, 512 * g:512 * (g + 1)],
                                  in_=l3sb[32 * g:32 * g + 27, :])

    nc.compile()
    return nc


def _get_nc():
    if "nc" not in _CACHE:
        _CACHE["nc"] = _build_bass()
    return _CACHE["nc"]


def kernel(globals_, locals_, gw1, gb1, gw2, gb2, gw3, gb3,
           lw1, lb1, lw2, lb2, lw3, lb3, idx_t):
    from concourse.bass_utils import run_bass_kernel_spmd

    globals_ = np.asarray(globals_, dtype=np.float32)
    locals_ = np.asarray(locals_, dtype=np.float32)
    idx = np.asarray(idx_t)
    f32 = np.float32

    ls = np.take_along_axis(locals_, idx[:, :, None].astype(np.int64), axis=1)
    ls = ls.reshape(B, MT * LD)

    gloT = np.ascontiguousarray(globals_.T, dtype=f32)          # [64, 96]
    z64 = np.zeros((64, B), f32)
    gloTa = np.concatenate([gloT, z64], axis=0)                  # rows 0..64
    gloTb = np.concatenate([z64, gloT], axis=0)                  # rows 64..128
    gw1a = np.ascontiguousarray(gw1[:MT * LD], f32)
    gw1bp = np.concatenate([np.asarray(gw1[MT * LD:], f32),
                            np.zeros((64, H), f32)], axis=0)
    w3dg = np.zeros((H, 32, 32), f32)
    w3dg[:, np.arange(32), np.arange(32)] = np.asarray(lw3, f32).reshape(H, 1)
    gw3d = np.zeros((H, 32), f32)
    gw3d[:, 0] = np.asarray(gw3, f32).ravel()

    shared = {
        "gloTa": gloTa, "gloTb": gloTb,
        "gw1a": gw1a, "gw1bp": gw1bp,
        "gw2": np.ascontiguousarray(gw2, f32),
        "lw1": np.ascontiguousarray(lw1, f32),
        "lw2": np.ascontiguousarray(lw2, f32),
        "w3dg": w3dg.reshape(H, 32 * 32), "gw3d": gw3d,
        "gb1": np.asarray(gb1, f32).reshape(H, 1),
        "gb2": np.asarray(gb2, f32).reshape(H, 1),
        "lb1": np.asarray(lb1, f32).reshape(H, 1),
        "lb2": np.asarray(lb2, f32).reshape(H, 1),
    }
    in_maps = []
    for c in range(NCORES):
        rows = slice(c * BI, (c + 1) * BI)
        lsT_c = np.ascontiguousarray(ls[rows].T, f32)            # [640, 12]
        locT_c = np.ascontiguousarray(
            locals_[rows].reshape(BI * T, LD).T, f32)            # [64, 576]
        locT_c = np.concatenate([locT_c, np.zeros((64, NC_COLS), f32)], axis=0)
        in_maps.append({**shared, "lsT": lsT_c, "locT": locT_c})

    nc = _get_nc()
    res = run_bass_kernel_spmd(nc, in_maps, core_ids=list(range(NCORES)))

    gb3s = float(np.asarray(gb3).ravel()[0])
    lb3s = float(np.asarray(lb3).ravel()[0])
    globals_mi = np.concatenate(
        [res.results[c]["gmi"].reshape(BI, B) for c in range(NCORES)], axis=0
    ) + gb3s
    locals_mi = np.concatenate(
        [res.results[c]["lmi"].reshape(B, BI, T).transpose(2, 1, 0)
         for c in range(NCORES)], axis=1) + lb3s

    return (globals_, locals_,
            globals_mi.astype(np.float32), np.ascontiguousarray(locals_mi, np.float32))


# revision 14
# speedup vs baseline: 1.0245x; 1.0245x over previous
"""Trainium2 Bass kernel for pairwise MI discriminator (gnn_message_passing).

Math (see reference):
  ls   = gather(locals_, idx_t)                       [B, MT*LD]
  globals_mi[i,j]   = MLPg(concat(ls[i], globals_[j]))          [B,B]
  locals_mi[t,i,j]  = MLPl(concat(locals_[i,t], globals_[j]))   [T,B,B]
where each MLP is Linear+ReLU, Linear+ReLU, Linear(->1).

Key factorization: layer 1 of each MLP splits over the concat:
  h1[i,j] = relu(A[i] + Bg[j] + b1),  A = ls @ W1[:640], Bg = globals_ @ W1[640:]
so the O(B^2) first-layer matmul collapses to O(B).

Sharding: rows i are split across 8 cores (12 rows each); globals_ and
weights replicated. Each core computes its [12,96] slice of globals_mi and
[96,12,48] slice of locals_mi ((j,i,t) order), host reassembles.

On-chip layout ("features on partitions"): activations are kept transposed
[128=hidden, cols] so layers 2/3 are natural matmuls with stationary weights.
Layer 3 (128->1) packs its per-column scalar outputs into PSUM partition rows
via a diagonal-w3 stationary operand + PSUM accumulation + 4-way column-group
tile_position concurrency, so one PSUM bank collects 64 j-columns of output
and is copied out in a single wide op.
"""

import sys

if "/opt/trn_rl_repo" not in sys.path:
    sys.path.insert(0, "/opt/trn_rl_repo")

import numpy as np

B, T, LD, GD, MT, H = 96, 48, 64, 64, 10, 128
NCORES = 8
BI = B // NCORES          # 12 rows of i per core
NC_COLS = BI * T          # 576 columns (i,t) per j
NPAIR = B // 2            # 48 j-pairs
Q = NC_COLS // 2          # 288 = quarter of a j-pair's 1152 cols

# engine split tables: combine per j (V=vector, G=gpsimd, A=scalar/ACT),
# psum-out per pair (A=scalar/ACT, V=vector)
CB_SPLIT = "V" * 48
PO_SPLIT = "A"

_CACHE = {}


def _build_bass():
    import concourse.mybir as mybir
    import concourse.tile as tile
    from concourse import bacc

    f32 = mybir.dt.float32
    f32r = mybir.dt.float32r
    bf16 = mybir.dt.bfloat16
    fp16 = mybir.dt.float16
    Alu = mybir.AluOpType
    Act = mybir.ActivationFunctionType

    nc = bacc.Bacc("TRN2", target_bir_lowering=False, debug=False)

    def din(name, shape):
        return nc.dram_tensor(name, shape, f32, kind="ExternalInput").ap()

    lsT = din("lsT", [MT * LD, BI])        # sampled locals, transposed
    locT = din("locT", [128, NC_COLS])     # locals rows this core, T-major, rows 64+ zero
    gloTa = din("gloTa", [128, B])         # globals_.T in rows 0..64, rest zero
    gloTb = din("gloTb", [128, B])         # globals_.T in rows 64..128, rest zero
    gw1a = din("gw1a", [MT * LD, H])
    gw1bp = din("gw1bp", [128, H])         # gw1[640:] zero-padded to 128 rows
    gw2d = din("gw2", [H, H])
    lw1d = din("lw1", [H, H])
    lw2d = din("lw2", [H, H])
    w3dg = din("w3dg", [H, 32 * 32])       # [h, r*32+m]: lw3[h] iff m==r
    gw3d = din("gw3d", [H, 32])            # [h, m]: gw3[h] iff m==0
    gb1 = din("gb1", [H, 1])
    gb2 = din("gb2", [H, 1])
    lb1 = din("lb1", [H, 1])
    lb2 = din("lb2", [H, 1])

    gmi = nc.dram_tensor("gmi", [BI * B], f32, kind="ExternalOutput").ap()
    lmi = nc.dram_tensor("lmi", [B, NC_COLS], f32, kind="ExternalOutput").ap()

    with tile.TileContext(nc) as tc:
        with (
            tc.tile_pool(name="const", bufs=1) as const,
            tc.tile_pool(name="h1p", bufs=4) as h1p,
            tc.tile_pool(name="h2p", bufs=4) as h2p,
            tc.tile_pool(name="l3p", bufs=2) as l3p,
            tc.tile_pool(name="ps2", bufs=2, space="PSUM") as ps2,
            tc.tile_pool(name="ps3", bufs=2, space="PSUM") as ps3,
        ):
            # ---- stage 0: load constants -------------------------------
            # spread loads over the per-engine DMA queues so they don't
            # serialize on one HWDGE ring
            _qs = [nc.sync, nc.scalar, nc.gpsimd]
            _qi = [0]

            def load(ap_, shape, rearr=None, **kw):
                t = const.tile(shape, f32, tag=f"c_{ap_.name}")
                eng = _qs[_qi[0] % len(_qs)]
                _qi[0] += 1
                eng.dma_start(out=t[:], in_=ap_ if rearr is None else ap_.rearrange(rearr, **kw))
                return t

            lw1_sb = load(lw1d, [H, H])
            locT_sb = load(locT, [128, NC_COLS])
            gloTb_sb = load(gloTb, [128, B])
            lb1_sb = load(lb1, [H, 1])
            lw2_sb = load(lw2d, [H, H])
            lb2_sb = load(lb2, [H, 1])
            lsT_sb = load(lsT, [128, 5, BI], "(o p) m -> p o m", p=128)
            gw1a_sb = load(gw1a, [128, 5, H], "(o p) m -> p o m", p=128)
            gloTa_sb = load(gloTa, [128, B])
            gw1bp_sb = load(gw1bp, [128, H])
            gw2_sb = load(gw2d, [H, H])
            gb1_sb = load(gb1, [H, 1])
            gb2_sb = load(gb2, [H, 1])
            gw3d_sb = load(gw3d, [H, 32])
            w3dg_sb = load(w3dg, [H, 32, 32], "h (r m) -> h r m", m=32)

            # ---- stage 0: first-layer factor matmuls -------------------
            # AlT [128,576] = lw1[:64].T @ locT  (+lb1)
            ps_al = ps2.tile([128, NC_COLS], f32, tag="ps2")
            nc.tensor.matmul(ps_al[:, 0:512], lw1_sb[:], locT_sb[:, 0:512], start=True, stop=True)
            nc.tensor.matmul(ps_al[:, 512:NC_COLS], lw1_sb[:], locT_sb[:, 512:NC_COLS],
                             start=True, stop=True)
            alT = const.tile([128, NC_COLS], f32)
            nc.vector.tensor_scalar_add(alT[:], ps_al[:], lb1_sb[:, 0:1])

            # BlT [128,96] = lw1[64:].T @ globals_.T
            ps_bl = ps2.tile([128, B], f32, tag="ps2")
            nc.tensor.matmul(ps_bl[:], lw1_sb[:], gloTb_sb[:], start=True, stop=True)
            blT = const.tile([128, B], f32)
            nc.vector.tensor_copy(blT[:], ps_bl[:])

            # AgT [128,12] = gw1[:640].T @ ls.T   (+gb1)
            ps_ag = ps2.tile([128, BI], f32, tag="ps2")
            for k in range(5):
                nc.tensor.matmul(ps_ag[:], gw1a_sb[:, k, :], lsT_sb[:, k, :],
                                 start=(k == 0), stop=(k == 4))
            agT = const.tile([128, BI], f32)
            nc.vector.tensor_scalar_add(agT[:], ps_ag[:], gb1_sb[:, 0:1])

            # BgT [128,96] = gw1[640:].T @ globals_.T
            ps_bg = ps2.tile([128, B], f32, tag="ps2")
            nc.tensor.matmul(ps_bg[:], gw1bp_sb[:], gloTa_sb[:], start=True, stop=True)
            bgT = const.tile([128, B], f32)
            nc.vector.tensor_copy(bgT[:], ps_bg[:])

            def mm_chunks(psum, lhsT, rhs, n):
                # split free dim into <=512 chunks aligned to PSUM banks
                o = 0
                while o < n:
                    w = min(512, n - o)
                    nc.tensor.matmul(psum[:, o:o + w], lhsT, rhs[:, o:o + w],
                                     start=True, stop=True)
                    o += w

            # ---- globals branch ---------------------------------------
            h1g = h1p.tile([128, BI * B], f32, tag="h1")
            for i in range(BI):
                nc.vector.tensor_scalar(h1g[:, i * B:(i + 1) * B], bgT[:],
                                        agT[:, i:i + 1], 0.0, op0=Alu.add, op1=Alu.max)
            ps_g2 = ps2.tile([128, BI * B], f32, tag="ps2")
            mm_chunks(ps_g2, gw2_sb[:], h1g, BI * B)
            h2g = h2p.tile([128, BI * B], f32, tag="h2")
            nc.scalar.activation(h2g[:], ps_g2[:], Act.Relu, bias=gb2_sb[:, 0:1])
            ps_g3 = ps3.tile([128, Q], f32, tag="ps3")
            for g in range(4):
                nc.tensor.matmul(ps_g3[32 * g:32 * (g + 1), :], gw3d_sb[:],
                                 h2g[:, Q * g:Q * (g + 1)], start=True, stop=True,
                                 tile_position=(0, 32 * g))
            g3sb = l3p.tile([128, Q], f32, tag="l3")
            nc.scalar.activation(g3sb[:], ps_g3[:], Act.Copy)
            nc.sync.dma_start(out=gmi.rearrange("(g c) -> g c", g=4),
                              in_=g3sb[0:128:32, :])

            # ---- locals branch: loop over j-pairs ----------------------
            # layer-3 outputs accumulate into ps_big banks: bank holds rows
            # 32g+r = (quarter g of pair r); bank 0 <- pairs 0..31, bank 1 <- 32..47
            banks = [(0, 32), (32, 16)]
            for bank_idx, (p0, nr) in enumerate(banks):
                ps_big = ps3.tile([128, Q], f32, tag="ps3")
                for r in range(nr):
                    p = p0 + r
                    j0 = 2 * p
                    h1l = h1p.tile([128, 2 * NC_COLS], f32, tag="h1")
                    for jj in range(2):
                        j = j0 + jj
                        dst = h1l[:, jj * NC_COLS:(jj + 1) * NC_COLS]
                        if CB_SPLIT[j % len(CB_SPLIT)] == "A":
                            nc.scalar.activation(dst, alT[:], Act.Relu,
                                                 bias=blT[:, j:j + 1])
                        else:
                            nc.vector.tensor_scalar(dst, alT[:], blT[:, j:j + 1], 0.0,
                                                    op0=Alu.add, op1=Alu.max)
                    ps_l2 = ps2.tile([128, 2 * NC_COLS], f32, tag="ps2")
                    mm_chunks(ps_l2, lw2_sb[:], h1l, 2 * NC_COLS)
                    h2l = h2p.tile([128, 2 * NC_COLS], f32, tag="h2")
                    if PO_SPLIT[p % len(PO_SPLIT)] == "A":
                        nc.scalar.activation(h2l[:], ps_l2[:], Act.Relu, bias=lb2_sb[:, 0:1])
                    else:
                        nc.vector.tensor_scalar(h2l[:], ps_l2[:], lb2_sb[:, 0:1], 0.0,
                                                op0=Alu.add, op1=Alu.max)
                    for g in range(4):
                        nc.tensor.matmul(ps_big[32 * g:32 * (g + 1), :],
                                         w3dg_sb[:, r, :], h2l[:, Q * g:Q * (g + 1)],
                                         start=(r == 0), stop=(r == nr - 1),
                                         tile_position=(0, 32 * g))
                l3sb = l3p.tile([128, Q], f32, tag="l3")
                nc.scalar.activation(l3sb[:], ps_big[:], Act.Copy)
                # row 32*(2jj+gl)+r  <->  lmi[2*(p0+r)+jj, 288*gl:...]
                for jj in range(2):
                    for gl in range(2):
                        g = 2 * jj + gl
                        nc.sync.dma_start(
                            out=lmi[2 * p0 + jj:2 * p0 + jj + 2 * nr - 1:2,
                                    Q * gl:Q * (gl + 1)],
                            in_=l3sb[32 * g:32 * g + nr, :])

    nc.compile()
    return nc


def _get_nc():
    if "nc" not in _CACHE:
        _CACHE["nc"] = _build_bass()
    return _CACHE["nc"]


def kernel(globals_, locals_, gw1, gb1, gw2, gb2, gw3, gb3,
           lw1, lb1, lw2, lb2, lw3, lb3, idx_t):
    from concourse.bass_utils import run_bass_kernel_spmd

    globals_ = np.asarray(globals_, dtype=np.float32)
    locals_ = np.asarray(locals_, dtype=np.float32)
    idx = np.asarray(idx_t)
    f32 = np.float32

    ls = np.take_along_axis(locals_, idx[:, :, None].astype(np.int64), axis=1)
    ls = ls.reshape(B, MT * LD)

    gloT = np.ascontiguousarray(globals_.T, dtype=f32)          # [64, 96]
    z64 = np.zeros((64, B), f32)
    gloTa = np.concatenate([gloT, z64], axis=0)                  # rows 0..64
    gloTb = np.concatenate([z64, gloT], axis=0)                  # rows 64..128
    gw1a = np.ascontiguousarray(gw1[:MT * LD], f32)
    gw1bp = np.concatenate([np.asarray(gw1[MT * LD:], f32),
                            np.zeros((64, H), f32)], axis=0)
    w3dg = np.zeros((H, 32, 32), f32)
    w3dg[:, np.arange(32), np.arange(32)] = np.asarray(lw3, f32).reshape(H, 1)
    gw3d = np.zeros((H, 32), f32)
    gw3d[:, 0] = np.asarray(gw3, f32).ravel()

    shared = {
        "gloTa": gloTa, "gloTb": gloTb,
        "gw1a": gw1a, "gw1bp": gw1bp,
        "gw2": np.ascontiguousarray(gw2, f32),
        "lw1": np.ascontiguousarray(lw1, f32),
        "lw2": np.ascontiguousarray(lw2, f32),
        "w3dg": w3dg.reshape(H, 32 * 32), "gw3d": gw3d,
        "gb1": np.asarray(gb1, f32).reshape(H, 1),
        "gb2": np.asarray(gb2, f32).reshape(H, 1),
        "lb1": np.asarray(lb1, f32).reshape(H, 1),
        "lb2": np.asarray(lb2, f32).reshape(H, 1),
    }
    in_maps = []
    for c in range(NCORES):
        rows = slice(c * BI, (c + 1) * BI)
        lsT_c = np.ascontiguousarray(ls[rows].T, f32)            # [640, 12]
        locT_c = np.ascontiguousarray(
            locals_[rows].reshape(BI * T, LD).T, f32)            # [64, 576]
        locT_c = np.concatenate([locT_c, np.zeros((64, NC_COLS), f32)], axis=0)
        in_maps.append({**shared, "lsT": lsT_c, "locT": locT_c})

    nc = _get_nc()
    res = run_bass_kernel_spmd(nc, in_maps, core_ids=list(range(NCORES)))

    gb3s = float(np.asarray(gb3).ravel()[0])
    lb3s = float(np.asarray(lb3).ravel()[0])
    globals_mi = np.concatenate(
        [res.results[c]["gmi"].reshape(BI, B) for c in range(NCORES)], axis=0
    ) + gb3s
    locals_mi = np.concatenate(
        [res.results[c]["lmi"].reshape(B, BI, T).transpose(2, 1, 0)
         for c in range(NCORES)], axis=1) + lb3s

    return (globals_, locals_,
            globals_mi.astype(np.float32), np.ascontiguousarray(locals_mi, np.float32))


# revision 15
# speedup vs baseline: 2.0223x; 1.9740x over previous
"""Trainium2 Bass kernel for pairwise MI discriminator (gnn_message_passing).

Math (see reference):
  ls   = gather(locals_, idx_t)                       [B, MT*LD]
  globals_mi[i,j]   = MLPg(concat(ls[i], globals_[j]))          [B,B]
  locals_mi[t,i,j]  = MLPl(concat(locals_[i,t], globals_[j]))   [T,B,B]
where each MLP is Linear+ReLU, Linear+ReLU, Linear(->1).

Key factorization: layer 1 of each MLP splits over the concat:
  h1[i,j] = relu(A[i] + Bg[j] + b1),  A = ls @ W1[:640], Bg = globals_ @ W1[640:]
so the O(B^2) first-layer matmul collapses to O(B).

Sharding: rows i are split across 8 cores (12 rows each); globals_ and
weights replicated. Each core computes its [12,96] slice of globals_mi and
[96,12,48] slice of locals_mi ((j,i,t) order), host reassembles.

On-chip layout ("features on partitions"): activations are kept transposed
[128=hidden, cols] so layers 2/3 are natural matmuls with stationary weights.
Layer 3 (128->1) packs its per-column scalar outputs into PSUM partition rows
via a diagonal-w3 stationary operand + PSUM accumulation + 4-way column-group
tile_position concurrency, so one PSUM bank collects 64 j-columns of output
and is copied out in a single wide op.
"""

import sys

if "/opt/trn_rl_repo" not in sys.path:
    sys.path.insert(0, "/opt/trn_rl_repo")

import numpy as np

B, T, LD, GD, MT, H = 96, 48, 64, 64, 10, 128
NCORES = 8
BI = B // NCORES          # 12 rows of i per core
NC_COLS = BI * T          # 576 columns (i,t) per j
NPAIR = B // 2            # 48 j-pairs
Q = NC_COLS // 2          # 288 = quarter of a j-pair's 1152 cols

# engine split tables: combine per j (V=vector, G=gpsimd, A=scalar/ACT),
# psum-out per pair (A=scalar/ACT, V=vector)
CB_SPLIT = "V" * 48
PO_SPLIT = "A"
# "exact": all matmuls fp32 (bit-accurate vs fp32 reference, rel err ~7e-7)
# "fast": layers 2/3 in fp16 (10-bit mantissa, rel err ~7e-4), ~2x faster
PRECISION = "fast"

_CACHE = {}


def _build_bass():
    import concourse.mybir as mybir
    import concourse.tile as tile
    from concourse import bacc

    f32 = mybir.dt.float32
    f32r = mybir.dt.float32r
    bf16 = mybir.dt.bfloat16
    fp16 = mybir.dt.float16
    Alu = mybir.AluOpType
    Act = mybir.ActivationFunctionType

    nc = bacc.Bacc("TRN2", target_bir_lowering=False, debug=False)

    def din(name, shape):
        return nc.dram_tensor(name, shape, f32, kind="ExternalInput").ap()

    lsT = din("lsT", [MT * LD, BI])        # sampled locals, transposed
    locT = din("locT", [128, NC_COLS])     # locals rows this core, T-major, rows 64+ zero
    gloTa = din("gloTa", [128, B])         # globals_.T in rows 0..64, rest zero
    gloTb = din("gloTb", [128, B])         # globals_.T in rows 64..128, rest zero
    gw1a = din("gw1a", [MT * LD, H])
    gw1bp = din("gw1bp", [128, H])         # gw1[640:] zero-padded to 128 rows
    gw2d = din("gw2", [H, H])
    lw1d = din("lw1", [H, H])
    lw2d = din("lw2", [H, H])
    w3dg = din("w3dg", [H, 32 * 32])       # [h, r*32+m]: lw3[h] iff m==r
    gw3d = din("gw3d", [H, 32])            # [h, m]: gw3[h] iff m==0
    gb1 = din("gb1", [H, 1])
    gb2 = din("gb2", [H, 1])
    lb1 = din("lb1", [H, 1])
    lb2 = din("lb2", [H, 1])

    gmi = nc.dram_tensor("gmi", [BI * B], f32, kind="ExternalOutput").ap()
    lmi = nc.dram_tensor("lmi", [B, NC_COLS], f32, kind="ExternalOutput").ap()

    with tile.TileContext(nc) as tc:
        with (
            tc.tile_pool(name="const", bufs=1) as const,
            tc.tile_pool(name="h1p", bufs=4) as h1p,
            tc.tile_pool(name="h2p", bufs=4) as h2p,
            tc.tile_pool(name="l3p", bufs=2) as l3p,
            tc.tile_pool(name="ps2", bufs=2, space="PSUM") as ps2,
            tc.tile_pool(name="ps3", bufs=2, space="PSUM") as ps3,
        ):
            # ---- stage 0: load constants -------------------------------
            # spread loads over the per-engine DMA queues so they don't
            # serialize on one HWDGE ring
            _qs = [nc.sync, nc.scalar, nc.gpsimd]
            _qi = [0]

            def load(ap_, shape, rearr=None, **kw):
                t = const.tile(shape, f32, tag=f"c_{ap_.name}")
                eng = _qs[_qi[0] % len(_qs)]
                _qi[0] += 1
                eng.dma_start(out=t[:], in_=ap_ if rearr is None else ap_.rearrange(rearr, **kw))
                return t

            lw1_sb = load(lw1d, [H, H])
            locT_sb = load(locT, [128, NC_COLS])
            gloTb_sb = load(gloTb, [128, B])
            lb1_sb = load(lb1, [H, 1])
            lw2_sb = load(lw2d, [H, H])
            lb2_sb = load(lb2, [H, 1])
            lsT_sb = load(lsT, [128, 5, BI], "(o p) m -> p o m", p=128)
            gw1a_sb = load(gw1a, [128, 5, H], "(o p) m -> p o m", p=128)
            gloTa_sb = load(gloTa, [128, B])
            gw1bp_sb = load(gw1bp, [128, H])
            gw2_sb = load(gw2d, [H, H])
            gb1_sb = load(gb1, [H, 1])
            gb2_sb = load(gb2, [H, 1])
            gw3d_sb = load(gw3d, [H, 32])
            w3dg_sb = load(w3dg, [H, 32, 32], "h (r m) -> h r m", m=32)

            # ---- stage 0: first-layer factor matmuls -------------------
            # AlT [128,576] = lw1[:64].T @ locT  (+lb1)
            ps_al = ps2.tile([128, NC_COLS], f32, tag="ps2")
            nc.tensor.matmul(ps_al[:, 0:512], lw1_sb[:], locT_sb[:, 0:512], start=True, stop=True)
            nc.tensor.matmul(ps_al[:, 512:NC_COLS], lw1_sb[:], locT_sb[:, 512:NC_COLS],
                             start=True, stop=True)
            alT = const.tile([128, NC_COLS], f32)
            nc.vector.tensor_scalar_add(alT[:], ps_al[:], lb1_sb[:, 0:1])

            # BlT [128,96] = lw1[64:].T @ globals_.T
            ps_bl = ps2.tile([128, B], f32, tag="ps2")
            nc.tensor.matmul(ps_bl[:], lw1_sb[:], gloTb_sb[:], start=True, stop=True)
            blT = const.tile([128, B], f32)
            nc.vector.tensor_copy(blT[:], ps_bl[:])

            # AgT [128,12] = gw1[:640].T @ ls.T   (+gb1)
            ps_ag = ps2.tile([128, BI], f32, tag="ps2")
            for k in range(5):
                nc.tensor.matmul(ps_ag[:], gw1a_sb[:, k, :], lsT_sb[:, k, :],
                                 start=(k == 0), stop=(k == 4))
            agT = const.tile([128, BI], f32)
            nc.vector.tensor_scalar_add(agT[:], ps_ag[:], gb1_sb[:, 0:1])

            # BgT [128,96] = gw1[640:].T @ globals_.T
            ps_bg = ps2.tile([128, B], f32, tag="ps2")
            nc.tensor.matmul(ps_bg[:], gw1bp_sb[:], gloTa_sb[:], start=True, stop=True)
            bgT = const.tile([128, B], f32)
            nc.vector.tensor_copy(bgT[:], ps_bg[:])

            if PRECISION == "fast":
                mmdt = fp16
                lw2_mm = const.tile([H, H], fp16, tag="lw2h")
                nc.vector.tensor_copy(lw2_mm[:], lw2_sb[:])
                gw2_mm = const.tile([H, H], fp16, tag="gw2h")
                nc.vector.tensor_copy(gw2_mm[:], gw2_sb[:])
                w3dg_mm = const.tile([H, 32, 32], fp16, tag="w3dgh")
                nc.vector.tensor_copy(w3dg_mm[:], w3dg_sb[:])
                gw3d_mm = const.tile([H, 32], fp16, tag="gw3dh")
                nc.vector.tensor_copy(gw3d_mm[:], gw3d_sb[:])
            else:
                mmdt = f32
                lw2_mm, gw2_mm, w3dg_mm, gw3d_mm = lw2_sb, gw2_sb, w3dg_sb, gw3d_sb

            def mm_chunks(psum, lhsT, rhs, n):
                # split free dim into <=512 chunks aligned to PSUM banks
                o = 0
                while o < n:
                    w = min(512, n - o)
                    nc.tensor.matmul(psum[:, o:o + w], lhsT, rhs[:, o:o + w],
                                     start=True, stop=True)
                    o += w

            # ---- globals branch ---------------------------------------
            h1g = h1p.tile([128, BI * B], mmdt, tag="h1")
            for i in range(BI):
                nc.vector.tensor_scalar(h1g[:, i * B:(i + 1) * B], bgT[:],
                                        agT[:, i:i + 1], 0.0, op0=Alu.add, op1=Alu.max)
            ps_g2 = ps2.tile([128, BI * B], f32, tag="ps2")
            mm_chunks(ps_g2, gw2_mm[:], h1g, BI * B)
            h2g = h2p.tile([128, BI * B], mmdt, tag="h2")
            nc.scalar.activation(h2g[:], ps_g2[:], Act.Relu, bias=gb2_sb[:, 0:1])
            ps_g3 = ps3.tile([128, Q], f32, tag="ps3")
            for g in range(4):
                nc.tensor.matmul(ps_g3[32 * g:32 * (g + 1), :], gw3d_mm[:],
                                 h2g[:, Q * g:Q * (g + 1)], start=True, stop=True,
                                 tile_position=(0, 32 * g))
            g3sb = l3p.tile([128, Q], f32, tag="l3")
            nc.scalar.activation(g3sb[:], ps_g3[:], Act.Copy)
            nc.sync.dma_start(out=gmi.rearrange("(g c) -> g c", g=4),
                              in_=g3sb[0:128:32, :])

            # ---- locals branch: loop over j-pairs ----------------------
            # layer-3 outputs accumulate into ps_big banks: bank holds rows
            # 32g+r = (quarter g of pair r); bank 0 <- pairs 0..31, bank 1 <- 32..47
            banks = [(0, 32), (32, 16)]
            for bank_idx, (p0, nr) in enumerate(banks):
                ps_big = ps3.tile([128, Q], f32, tag="ps3")
                for r in range(nr):
                    p = p0 + r
                    j0 = 2 * p
                    h1l = h1p.tile([128, 2 * NC_COLS], mmdt, tag="h1")
                    for jj in range(2):
                        j = j0 + jj
                        dst = h1l[:, jj * NC_COLS:(jj + 1) * NC_COLS]
                        if CB_SPLIT[j % len(CB_SPLIT)] == "A":
                            nc.scalar.activation(dst, alT[:], Act.Relu,
                                                 bias=blT[:, j:j + 1])
                        else:
                            nc.vector.tensor_scalar(dst, alT[:], blT[:, j:j + 1], 0.0,
                                                    op0=Alu.add, op1=Alu.max)
                    ps_l2 = ps2.tile([128, 2 * NC_COLS], f32, tag="ps2")
                    mm_chunks(ps_l2, lw2_mm[:], h1l, 2 * NC_COLS)
                    h2l = h2p.tile([128, 2 * NC_COLS], mmdt, tag="h2")
                    if PO_SPLIT[p % len(PO_SPLIT)] == "A":
                        nc.scalar.activation(h2l[:], ps_l2[:], Act.Relu, bias=lb2_sb[:, 0:1])
                    else:
                        nc.vector.tensor_scalar(h2l[:], ps_l2[:], lb2_sb[:, 0:1], 0.0,
                                                op0=Alu.add, op1=Alu.max)
                    for g in range(4):
                        nc.tensor.matmul(ps_big[32 * g:32 * (g + 1), :],
                                         w3dg_mm[:, r, :], h2l[:, Q * g:Q * (g + 1)],
                                         start=(r == 0), stop=(r == nr - 1),
                                         tile_position=(0, 32 * g))
                l3sb = l3p.tile([128, Q], f32, tag="l3")
                nc.scalar.activation(l3sb[:], ps_big[:], Act.Copy)
                # row 32*(2jj+gl)+r  <->  lmi[2*(p0+r)+jj, 288*gl:...]
                for jj in range(2):
                    for gl in range(2):
                        g = 2 * jj + gl
                        nc.sync.dma_start(
                            out=lmi[2 * p0 + jj:2 * p0 + jj + 2 * nr - 1:2,
                                    Q * gl:Q * (gl + 1)],
                            in_=l3sb[32 * g:32 * g + nr, :])

    nc.compile()
    return nc


def _get_nc():
    if "nc" not in _CACHE:
        _CACHE["nc"] = _build_bass()
    return _CACHE["nc"]


def kernel(globals_, locals_, gw1, gb1, gw2, gb2, gw3, gb3,
           lw1, lb1, lw2, lb2, lw3, lb3, idx_t):
    from concourse.bass_utils import run_bass_kernel_spmd

    globals_ = np.asarray(globals_, dtype=np.float32)
    locals_ = np.asarray(locals_, dtype=np.float32)
    idx = np.asarray(idx_t)
    f32 = np.float32

    ls = np.take_along_axis(locals_, idx[:, :, None].astype(np.int64), axis=1)
    ls = ls.reshape(B, MT * LD)

    gloT = np.ascontiguousarray(globals_.T, dtype=f32)          # [64, 96]
    z64 = np.zeros((64, B), f32)
    gloTa = np.concatenate([gloT, z64], axis=0)                  # rows 0..64
    gloTb = np.concatenate([z64, gloT], axis=0)                  # rows 64..128
    gw1a = np.ascontiguousarray(gw1[:MT * LD], f32)
    gw1bp = np.concatenate([np.asarray(gw1[MT * LD:], f32),
                            np.zeros((64, H), f32)], axis=0)
    w3dg = np.zeros((H, 32, 32), f32)
    w3dg[:, np.arange(32), np.arange(32)] = np.asarray(lw3, f32).reshape(H, 1)
    gw3d = np.zeros((H, 32), f32)
    gw3d[:, 0] = np.asarray(gw3, f32).ravel()

    shared = {
        "gloTa": gloTa, "gloTb": gloTb,
        "gw1a": gw1a, "gw1bp": gw1bp,
        "gw2": np.ascontiguousarray(gw2, f32),
        "lw1": np.ascontiguousarray(lw1, f32),
        "lw2": np.ascontiguousarray(lw2, f32),
        "w3dg": w3dg.reshape(H, 32 * 32), "gw3d": gw3d,
        "gb1": np.asarray(gb1, f32).reshape(H, 1),
        "gb2": np.asarray(gb2, f32).reshape(H, 1),
        "lb1": np.asarray(lb1, f32).reshape(H, 1),
        "lb2": np.asarray(lb2, f32).reshape(H, 1),
    }
    in_maps = []
    for c in range(NCORES):
        rows = slice(c * BI, (c + 1) * BI)
        lsT_c = np.ascontiguousarray(ls[rows].T, f32)            # [640, 12]
        locT_c = np.ascontiguousarray(
            locals_[rows].reshape(BI * T, LD).T, f32)            # [64, 576]
        locT_c = np.concatenate([locT_c, np.zeros((64, NC_COLS), f32)], axis=0)
        in_maps.append({**shared, "lsT": lsT_c, "locT": locT_c})

    nc = _get_nc()
    res = run_bass_kernel_spmd(nc, in_maps, core_ids=list(range(NCORES)))

    gb3s = float(np.asarray(gb3).ravel()[0])
    lb3s = float(np.asarray(lb3).ravel()[0])
    globals_mi = np.concatenate(
        [res.results[c]["gmi"].reshape(BI, B) for c in range(NCORES)], axis=0
    ) + gb3s
    locals_mi = np.concatenate(
        [res.results[c]["lmi"].reshape(B, BI, T).transpose(2, 1, 0)
         for c in range(NCORES)], axis=1) + lb3s

    return (globals_, locals_,
            globals_mi.astype(np.float32), np.ascontiguousarray(locals_mi, np.float32))
